# revision 25
# baseline (speedup 1.0000x reference)
"""TF-IDF document model (histogram_binning) on 8 TRN2 NeuronCores.

Algorithm (per core, 64 batch rows, data-parallel over batch):
  For each row b: tf-idf histogram over vocab V=50257 computed as a radix
  one-hot matmul on the PE: vocab index v = hi*393 + lo with hi in
  [0,128), lo in [0,393). For each 128-token chunk of the row,
  A[s,hi] = (hi_s == hi) (one-hot), B[s,lo] = (lo_s == lo)*idf[x_s]
  (idf-weighted one-hot, fused via the two-scalar tensor_scalar), and
  C[hi,lo] += A^T @ B accumulates the tf*idf matrix directly in PSUM.
  A-builds are split between the DVE and GpSimd engines to balance load;
  B-builds run on the DVE.
  Row sums n_b = sum_s idf[x_{b,s}] come from a single ones-matmul over
  the host-gathered idf values; the Act engine applies 1/n_b while
  casting C from PSUM fp32 to the fp16 output tile.
Output written as [64, 50304] fp16 per core (vocab padded 50257->50304);
host casts to fp32 and slices/concats to (512, 50257).
"""
import numpy as np

import concourse.bacc as bacc
import concourse.mybir as mybir
from concourse import bass_utils
from concourse.tile import TileContext

B, S, V = 512, 1024, 50257
NC = 8
BL = B // NC          # 64 rows per core
HI, LO = 128, 393     # radix split: v = hi*LO + lo
VP = HI * LO          # 50304 padded vocab
CH = S // 128         # 8 token chunks per row
GROUP = 8             # rows per output-DMA group

_cache = {}


def _build(repeat: int = 0, feat: str = "full"):
    nc = bacc.Bacc(
        "TRN2",
        target_bir_lowering=False,
        debug=False,
        enable_asserts=False,
        num_devices=NC,
    )
    hif_t = nc.dram_tensor("hif", [128, BL * CH], mybir.dt.float32, kind="ExternalInput")
    lof_t = nc.dram_tensor("lof", [128, BL * CH], mybir.dt.float32, kind="ExternalInput")
    idfg_t = nc.dram_tensor("idfg", [128, BL * CH], mybir.dt.float32, kind="ExternalInput")
    qpack_t = nc.dram_tensor("qpack", [128, 3 * GROUP * CH], mybir.dt.float32, kind="ExternalInput")
    # transposed layout: out[p, b*LO+f] = row b, vocab p*LO+f (host unshuffles)
    out_t = nc.dram_tensor("out", [128, BL * LO], mybir.dt.float16, kind="ExternalOutput")
    # half-group (4-row) output slices for finer write-out granularity
    ovh = out_t.ap().rearrange("p (g c) -> g p c", g=BL // (GROUP // 2))
    ovr = out_t.ap().rearrange("p (g c) -> g p c", g=BL)

    AF = mybir.ActivationFunctionType
    OP = mybir.AluOpType
    ncols = BL * CH

    with TileContext(nc) as tc:
        with (
            tc.tile_pool(name="const", bufs=1) as cpool,
            tc.tile_pool(name="work", bufs=4) as wpool,
            tc.tile_pool(name="ab", bufs=32) as abpool,
            tc.tile_pool(name="tt", bufs=5) as tpool,
            tc.tile_pool(name="ps", bufs=7, space="PSUM") as pspool,
            tc.tile_pool(name="ps2", bufs=1, space="PSUM") as ps2pool,
        ):
            # constants generated on-device: no DMA slots spent on them
            iota = cpool.tile([128, LO], mybir.dt.float16, tag="iota")
            nc.gpsimd.iota(
                out=iota[:], pattern=[[1, LO]], base=0, channel_multiplier=0,
                allow_small_or_imprecise_dtypes=True,
            )
            onesc = cpool.tile([128, 1], mybir.dt.float32, tag="onesc")
            nc.vector.memset(onesc[:], 1.0)
            onesr = cpool.tile([1, 128], mybir.dt.float32, tag="onesr")
            nc.vector.memset(onesr[:], 1.0)

            # big loads split in halves across two HWDGE queues (SP + Act) so
            # the first rows' columns land early
            HC = ncols // 2
            hifh = [cpool.tile([128, HC], mybir.dt.float32, tag=f"hif{h}",
                               name=f"hif{h}") for h in range(2)]
            lofh = [cpool.tile([128, HC], mybir.dt.float32, tag=f"lof{h}",
                               name=f"lof{h}") for h in range(2)]
            idfgh = [cpool.tile([128, HC], mybir.dt.float32, tag=f"idfg{h}",
                                name=f"idfg{h}") for h in range(2)]
            QC = GROUP * CH
            qpk = cpool.tile([128, 3 * QC], mybir.dt.float32, tag="qpk")
            nc.sync.dma_start(out=qpk[:], in_=qpack_t.ap())
            nc.sync.dma_start(out=idfgh[0][:], in_=idfg_t.ap()[:, :HC])
            nc.scalar.dma_start(out=lofh[0][:], in_=lof_t.ap()[:, :HC])
            nc.sync.dma_start(out=hifh[0][:], in_=hif_t.ap()[:, :HC])
            nc.scalar.dma_start(out=hifh[1][:], in_=hif_t.ap()[:, HC:])
            nc.sync.dma_start(out=lofh[1][:], in_=lof_t.ap()[:, HC:])
            nc.scalar.dma_start(out=idfgh[1][:], in_=idfg_t.ap()[:, HC:])

            def main_body(_iv=None):
                HB = BL // 2
                rbs = []

                def rb_chain(src, nrows, nm):
                    # row-sum reciprocals: n[b] = sum_s idfg via ones-matmul,
                    # then 1/n broadcast across partitions
                    psb = ps2pool.tile([128, 288], mybir.dt.float32, tag="psb",
                                       name=f"psb{nm}")
                    ncol = nrows * CH
                    n_ps = psb[0:1, 32:32 + ncol]
                    rb_ps = psb[:, 0:32]
                    nc.tensor.matmul(
                        out=n_ps, lhsT=onesc[:], rhs=src,
                        start=True, stop=True,
                    )
                    nsum = wpool.tile([1, HB], mybir.dt.float32, tag="nsum")
                    nc.vector.tensor_reduce(
                        out=nsum[:, :nrows],
                        in_=n_ps.rearrange("p (b c) -> p b c", c=CH),
                        axis=mybir.AxisListType.X,
                        op=OP.add,
                    )
                    recip = wpool.tile([1, HB], mybir.dt.float32, tag="recip")
                    nc.vector.reciprocal(out=recip[:, :nrows], in_=nsum[:, :nrows])
                    nc.tensor.matmul(
                        out=rb_ps[:, :nrows], lhsT=onesr[:], rhs=recip[:, :nrows],
                        start=True, stop=True,
                    )
                    rb = wpool.tile([128, HB], mybir.dt.float32, tag=f"rb{nm}",
                                    name=f"rb{nm}")
                    nc.vector.tensor_copy(out=rb[:, :nrows], in_=rb_ps[:, :nrows])
                    return rb

                rbq = rb_chain(qpk[:, 2 * QC :], GROUP, "q")

                HG = GROUP // 2
                for g in range(BL // GROUP):
                    Tg = tpool.tile([128, GROUP * LO], mybir.dt.float16, tag="Tg")
                    for r in range(GROUP):
                        row = g * GROUP + r
                        # A-builds on GpSimd per row; last rows lean on GpSimd
                        # (it drains earlier than the DVE otherwise)
                        npool = 7 if row >= 58 else 5 + (row % 2)
                        C = pspool.tile([HI, LO], mybir.dt.float32, tag="C")
                        As = []
                        for c in range(CH):
                            col = row * CH + c
                            if col < QC:
                                hsrc, cc = qpk[:, 0:QC], col
                            else:
                                h, cc = divmod(col, HC)
                                hsrc = hifh[h][:]
                            A = abpool.tile([128, HI], mybir.dt.float16, tag="A")
                            eng = nc.gpsimd if c < npool else nc.vector
                            eng.tensor_scalar(
                                out=A[:],
                                in0=iota[:, :HI],
                                scalar1=hsrc[:, cc : cc + 1],
                                scalar2=None,
                                op0=OP.is_equal,
                            )
                            As.append(A)
                        for c in range(CH):
                            col = row * CH + c
                            if col < QC:
                                lsrc, isrc, cc = qpk[:, QC : 2 * QC], qpk[:, 2 * QC :], col
                            else:
                                h, cc = divmod(col, HC)
                                lsrc, isrc = lofh[h][:], idfgh[h][:]
                            Bt = abpool.tile([128, LO], mybir.dt.float16, tag="B")
                            nc.vector.tensor_scalar(
                                out=Bt[:],
                                in0=iota[:],
                                scalar1=lsrc[:, cc : cc + 1],
                                scalar2=isrc[:, cc : cc + 1],
                                op0=OP.is_equal,
                                op1=OP.mult,
                            )
                            nc.tensor.matmul(
                                out=C[:],
                                lhsT=As[c][:],
                                rhs=Bt[:],
                                start=(c == 0),
                                stop=(c == CH - 1),
                            )
                        if row < GROUP:
                            rbh, ri = rbq, row
                        else:
                            rbh, ri = rbs[row // HB], row % HB
                        nc.scalar.activation(
                            out=Tg[:, r * LO : (r + 1) * LO],
                            in_=C[:],
                            func=AF.Copy,
                            scale=rbh[:, ri : ri + 1],
                        )
                        if g == 0 and r == 6:
                            rbs.append(rb_chain(idfgh[0][:], HB, "0"))
                        if feat != "nodma" and r == HG - 1:
                            nc.sync.dma_start(
                                out=ovh[2 * g], in_=Tg[:, : HG * LO]
                            )
                        if feat != "nodma" and g == BL // GROUP - 1 and r >= HG:
                            # last rows: per-row DMA so the tail drains early
                            nc.sync.dma_start(
                                out=ovr[row], in_=Tg[:, r * LO : (r + 1) * LO]
                            )
                    if feat == "nodma":
                        nc.vector.tensor_copy(out=rbs[0][:, :1], in_=Tg[:, :1])
                    elif g == 0:
                        nc.sync.dma_start(out=ovh[2 * g + 1], in_=Tg[:, HG * LO :])
                        rbs.append(rb_chain(idfgh[1][:], HB, "1"))
                    elif g < BL // GROUP - 1:
                        nc.sync.dma_start(out=ovh[2 * g + 1], in_=Tg[:, HG * LO :])

            if repeat:
                tc.For_i_unrolled(0, repeat, 1, main_body, max_unroll=1)
            else:
                main_body()
    nc.compile()
    return nc


def _get_nc():
    if "nc" not in _cache:
        _cache["nc"] = _build()
    return _cache["nc"]


def _host_inputs(x: np.ndarray, idf: np.ndarray):
    """Build per-core input maps from the full inputs."""
    idf32 = np.asarray(idf, dtype=np.float32)
    xi = np.asarray(x, dtype=np.int32)  # values < 2**31, safe cast
    hi_all = (xi // LO).astype(np.float32)
    lo_all = (xi % LO).astype(np.float32)
    idfg_all = idf32[xi]  # (B, S) gathered idf per token
    in_maps = []
    for k in range(NC):
        # layout [128, BL*CH]: element [p, b*CH+c] = v[b, c*128+p]
        def lay(a):
            ac = a[k * BL : (k + 1) * BL]
            return np.ascontiguousarray(
                ac.reshape(BL, CH, 128).transpose(2, 0, 1).reshape(128, BL * CH)
            )
        lh, ll, li = lay(hi_all), lay(lo_all), lay(idfg_all)
        QC = GROUP * CH
        in_maps.append(
            {
                "hif": lh,
                "lof": ll,
                "idfg": li,
                "qpack": np.ascontiguousarray(
                    np.concatenate([lh[:, :QC], ll[:, :QC], li[:, :QC]], axis=1)
                ),
            }
        )
    return in_maps


def kernel(x: np.ndarray, idf: np.ndarray) -> np.ndarray:
    nc = _get_nc()
    in_maps = _host_inputs(x, idf)
    res = bass_utils.run_bass_kernel_spmd(nc, in_maps, core_ids=list(range(NC)))
    outs = []
    for r in res.results:
        a = r["out"].astype(np.float32).reshape(128, BL, LO).transpose(1, 0, 2).reshape(BL, VP)
        outs.append(a[:, :V])
    return np.concatenate(outs, axis=0)
